# revision 4
# baseline (speedup 1.0000x reference)
"""Trainium2 Bass kernel for a custom LSTM cell.

Math (per reference):
    i = sigmoid(x @ W_i.T + b_Wi + h @ U_i.T + b_Ui)
    f = sigmoid(x @ W_f.T + b_Wf + h @ U_f.T + b_Uf + boundary @ W_b.T + b_Wb)
    o = sigmoid(x @ W_o.T + b_Wo + h @ U_o.T + b_Uo)
    g = tanh   (x @ W_g.T + b_Wg + h @ U_g.T + b_Ug)
    c = f * c_prev + i * g
    h = o * tanh(c)

Strategy: data-parallel over batch across 8 NeuronCores (1024 rows each).
Host-side we build A.T = [x | h_prev].T (K=1536 on partitions) and a single
fused weight matrix M [1536, 4096] whose columns are ordered per 256-wide
h-slice as [i | f | o | g], so the device only does natural-layout DMAs and
K-partition matmuls. Bias + boundary enter as one extra K=3 matmul step
(lhsT rows = [ones, boundary0, boundary1]).
"""

import sys

sys.path.insert(0, "/opt/trn_rl_repo")

import numpy as np

B, IN, H = 8192, 512, 1024
NCORES = 8
BLOC = B // NCORES  # 1024 rows per core
KTOT = IN + H  # 1536 contraction
KT = KTOT // 128  # 12 k-tiles
BT = BLOC // 128  # 8 batch tiles per core
SLICE = 256  # h-slice width per gate
NS = H // SLICE  # 4 slices
GW = 4 * SLICE  # 1024 columns of M per slice (i|f|o|g)

_PROG = None  # cached (nc, names) so repeat calls skip rebuild/recompile


def _build_program():
    import concourse.bass as bass
    import concourse.mybir as mybir
    import concourse.tile as tile
    from concourse import bacc
    from contextlib import ExitStack

    f32 = mybir.dt.float32
    f32r = mybir.dt.float32r
    SIG = mybir.ActivationFunctionType.Sigmoid
    TANH = mybir.ActivationFunctionType.Tanh

    nc = bacc.Bacc("TRN2", target_bir_lowering=False, debug=False)

    at_d = nc.dram_tensor("at_in", [KTOT, BLOC], f32, kind="ExternalInput").ap()
    et_d = nc.dram_tensor("et_in", [3, BLOC], f32, kind="ExternalInput").ap()
    m_d = nc.dram_tensor("m_in", [KTOT, 4 * H], f32, kind="ExternalInput").ap()
    r_d = nc.dram_tensor("r_in", [3, 4 * H], f32, kind="ExternalInput").ap()
    c_d = nc.dram_tensor("c_in", [BLOC, H], f32, kind="ExternalInput").ap()
    h_o = nc.dram_tensor("h_out", [BLOC, H], f32, kind="ExternalOutput").ap()
    c_o = nc.dram_tensor("c_out", [BLOC, H], f32, kind="ExternalOutput").ap()

    with tile.TileContext(nc) as tc:
        with ExitStack() as ctx:
            atp = ctx.enter_context(tc.tile_pool(name="atp", bufs=1))
            mp = ctx.enter_context(tc.tile_pool(name="mp", bufs=2))
            cst = ctx.enter_context(tc.tile_pool(name="cst", bufs=1))
            cinp = ctx.enter_context(tc.tile_pool(name="cinp", bufs=4))
            actp = ctx.enter_context(tc.tile_pool(name="actp", bufs=3))
            outp = ctx.enter_context(tc.tile_pool(name="outp", bufs=4))
            psp = ctx.enter_context(tc.tile_pool(name="psp", bufs=8, space="PSUM"))

            # Persistent activations: A.T as 12 [128, 1024] tiles
            at_tiles = []
            for k in range(KT):
                t = atp.tile([128, BLOC], f32r, name=f"at{k}", tag=f"at{k}")
                nc.gpsimd.dma_start(out=t, in_=at_d[k * 128 : (k + 1) * 128, :])
                at_tiles.append(t)

            et_t = cst.tile([3, BLOC], f32r, name="et_t")
            nc.gpsimd.dma_start(out=et_t, in_=et_d[:, :])
            r_t = cst.tile([3, 4 * H], f32r, name="r_t")
            nc.gpsimd.dma_start(out=r_t, in_=r_d[:, :])

            for s in range(NS):
                # weight tiles for this h-slice: columns [i|f|o|g] x SLICE
                m_tiles = []
                for k in range(KT):
                    t = mp.tile([128, GW], f32r, name=f"m{k}_{s}", tag=f"m{k}")
                    nc.gpsimd.dma_start(
                        out=t, in_=m_d[k * 128 : (k + 1) * 128, s * GW : (s + 1) * GW]
                    )
                    m_tiles.append(t)

                for b in range(BT):
                    bs = slice(b * 128, (b + 1) * 128)
                    ps_if = psp.tile([128, 512], f32, name=f"psif{s}_{b}", tag="ps")
                    ps_og = psp.tile([128, 512], f32, name=f"psog{s}_{b}", tag="ps")
                    for k in range(KT):
                        lhs = at_tiles[k][:, bs]
                        nc.tensor.matmul(
                            ps_if,
                            lhs,
                            m_tiles[k][:, 0:512],
                            start=(k == 0),
                            stop=False,
                        )
                        nc.tensor.matmul(
                            ps_og,
                            lhs,
                            m_tiles[k][:, 512:1024],
                            start=(k == 0),
                            stop=False,
                        )
                    # bias + boundary: K=3 step, rows [ones, bdry0, bdry1]
                    elhs = et_t[:, bs]
                    nc.tensor.matmul(
                        ps_if,
                        elhs,
                        r_t[:, s * GW : s * GW + 512],
                        start=False,
                        stop=True,
                    )
                    nc.tensor.matmul(
                        ps_og,
                        elhs,
                        r_t[:, s * GW + 512 : (s + 1) * GW],
                        start=False,
                        stop=True,
                    )

                    # gate nonlinearities (i,f -> sigmoid; o -> sigmoid; g -> tanh)
                    if_t = actp.tile([128, 512], f32, name=f"if{s}_{b}", tag="if")
                    og_t = actp.tile([128, 512], f32, name=f"og{s}_{b}", tag="og")
                    nc.scalar.activation(if_t, ps_if, SIG)
                    nc.scalar.activation(og_t[:, 0:SLICE], ps_og[:, 0:SLICE], SIG)
                    nc.scalar.activation(og_t[:, SLICE:512], ps_og[:, SLICE:512], TANH)

                    c_t = cinp.tile([128, SLICE], f32, name=f"cin{s}_{b}", tag="cin")
                    nc.sync.dma_start(
                        out=c_t, in_=c_d[bs, s * SLICE : (s + 1) * SLICE]
                    )

                    cn = outp.tile([128, SLICE], f32, name=f"cn{s}_{b}", tag="cn")
                    tmp = actp.tile([128, SLICE], f32, name=f"tmp{s}_{b}", tag="tmp")
                    # c' = f*c_prev + i*g
                    nc.vector.tensor_mul(cn, if_t[:, SLICE:512], c_t)
                    nc.vector.tensor_mul(tmp, if_t[:, 0:SLICE], og_t[:, SLICE:512])
                    nc.vector.tensor_add(cn, cn, tmp)
                    th = actp.tile([128, SLICE], f32, name=f"th{s}_{b}", tag="th")
                    nc.scalar.activation(th, cn, TANH)
                    hn = outp.tile([128, SLICE], f32, name=f"hn{s}_{b}", tag="hn")
                    nc.vector.tensor_mul(hn, og_t[:, 0:SLICE], th)

                    nc.sync.dma_start(
                        out=c_o[bs, s * SLICE : (s + 1) * SLICE], in_=cn
                    )
                    nc.sync.dma_start(
                        out=h_o[bs, s * SLICE : (s + 1) * SLICE], in_=hn
                    )
    nc.compile()
    return nc


def _get_program():
    global _PROG
    if _PROG is None:
        _PROG = _build_program()
    return _PROG


def _prep_inputs(inputs):
    """Host-side marshalling: fused weight matrix + transposed activations."""
    f = np.float32
    x = np.asarray(inputs["x"], f)
    h_prev = np.asarray(inputs["h_prev"], f)
    c_prev = np.asarray(inputs["c_prev"], f)
    boundary = np.asarray(inputs["boundary"], f)

    gates = ["i", "f", "o", "g"]
    W = {z: np.asarray(inputs[f"W_{z}"], f) for z in gates}
    U = {z: np.asarray(inputs[f"U_{z}"], f) for z in gates}
    bias = {
        z: np.asarray(inputs[f"b_W{z}"], f) + np.asarray(inputs[f"b_U{z}"], f)
        for z in gates
    }
    W_b = np.asarray(inputs["W_b"], f)
    b_Wb = np.asarray(inputs["b_Wb"], f)

    # M [1536, 4096]: rows 0-511 W.T, rows 512-1535 U.T; columns ordered per
    # 256-wide h-slice as [i | f | o | g].
    M = np.empty((KTOT, 4 * H), f)
    R = np.zeros((3, 4 * H), f)  # row0 bias; rows 1-2 boundary weights (f only)
    for s in range(NS):
        hs = slice(s * SLICE, (s + 1) * SLICE)
        for zi, z in enumerate(gates):
            cs = slice(s * GW + zi * SLICE, s * GW + (zi + 1) * SLICE)
            M[:IN, cs] = W[z][hs].T
            M[IN:, cs] = U[z][hs].T
            R[0, cs] = bias[z][hs]
            if z == "f":
                R[0, cs] += b_Wb[hs]
                R[1:3, cs] = W_b[hs].T

    AT = np.concatenate([x, h_prev], axis=1).T  # [1536, 8192]
    ET = np.concatenate(
        [np.ones((1, B), f), boundary.T.astype(f)], axis=0
    )  # [3, 8192]

    in_maps = []
    for c in range(NCORES):
        rs = slice(c * BLOC, (c + 1) * BLOC)
        in_maps.append(
            {
                "at_in": np.ascontiguousarray(AT[:, rs]),
                "et_in": np.ascontiguousarray(ET[:, rs]),
                "m_in": M,
                "r_in": R,
                "c_in": np.ascontiguousarray(c_prev[rs]),
            }
        )
    return in_maps


def run(inputs, trace=False):
    """Returns ((h, c), BassKernelResults)."""
    from concourse.bass_utils import run_bass_kernel_spmd

    nc = _get_program()
    in_maps = _prep_inputs(inputs)
    res = run_bass_kernel_spmd(
        nc, in_maps, core_ids=list(range(NCORES)), trace=trace
    )
    h = np.concatenate([r["h_out"] for r in res.results], axis=0)
    c = np.concatenate([r["c_out"] for r in res.results], axis=0)
    return (h, c), res


def kernel(**inputs):
    out, _ = run(inputs, trace=False)
    return out
